# revision 10
# baseline (speedup 1.0000x reference)
"""Distributed single-head attention on 8 TRN2 NeuronCores — zero-collective.

Math (matches the reference):
    q = z @ Wq; k = z @ Wk; v = z @ Wv
    out = softmax(q k^T) * DK**-0.5 @ v

Key idea: every core receives the FULL z (inputs are full-size anyway), so
k and v never need to be materialized or all-gathered.  Using associativity:
    S_r  = q_r k^T = (z_r Wq) Wk^T z^T      -> B^T = Wk q_r^T, S^T = z B^T
    out_r = P_r v  = (P_r z) Wv             -> C^T = z^T-accum of P^T, out = C^T^T Wv
Per-core FLOPs are identical to the gather-based flash schedule (736 unit
matmuls), but there are NO collectives: no skew-absorbing barrier, no
serialized CC stream, no PE stall waiting for gathered K/V (the baseline
idled the PE 38us there, which also re-throttled the HAM clock gate).

Sharding: core c processes rows [512c, 512c+512).  Host ships z rolled so
each core's own block is first: zT_roll (d-major, for S) and zn_roll
(seq-major, for C).  A j-tile index in the kernel is the global row
(c*512 + 128j) mod 4096 — a pure permutation, harmless under the j-sums.

Phases (all PE-dense, back-to-back):
    q^T (64 MM) -> B^T (64) -> S^T/exp/rowsum (256+32) -> C^T (256) -> out (64)
DMA: ~22MB of params per core at ~265GB/s on the sync ring (zT, weights),
zn streamed on the gpsimd ring during the C pass, Wv + rowsum round-trip on
the scalar ring.  PSUM: 8 banks for projections, 2+1 for S/rowsum, 8 for
C^T, 8 for out — sequential scopes.

Precision: fp16 z/W/q/B + f32 PSUM keeps logits to ~6e-3 abs err; exp and
C^T in bf16 (range: logits can reach ~70 pre-shift, so exp(S-40) can hit
e^30 — fp16 would overflow, bf16 is safe).  End-to-end rel err ~3e-3.
"""

import numpy as np

SEQ, D, DK, DV = 4096, 1024, 1024, 1024
NCORES = 8
ROWS = SEQ // NCORES            # 512 rows per core
DT = D // 128                   # 8 contraction tiles (input dim)
MT = DK // 128                  # 8 dk tiles
ST = ROWS // 128                # 4 local seq tiles
JT = SEQ // 128                 # 32 global seq tiles
SHIFT = 40.0                    # constant logit shift (softmax-invariant)
SCALE = DK ** -0.5


def _build():
    import concourse.mybir as mybir
    import concourse.tile as tile
    from concourse import bacc

    F32 = mybir.dt.float32
    F16 = mybir.dt.float16
    BF16 = mybir.dt.bfloat16
    Exp = mybir.ActivationFunctionType.Exp

    nc = bacc.Bacc("TRN2", target_bir_lowering=False, debug=False, num_devices=NCORES)
    d_zT = nc.declare_dram_parameter("zT", [D, SEQ], F16, isOutput=False)
    d_zn = nc.declare_dram_parameter("zn", [SEQ, D], F16, isOutput=False)
    d_wq = nc.declare_dram_parameter("Wq", [D, DK], F16, isOutput=False)
    d_wkt = nc.declare_dram_parameter("WkT", [DK, D], F16, isOutput=False)
    d_wv = nc.declare_dram_parameter("Wv", [D, DV], F16, isOutput=False)
    d_out = nc.declare_dram_parameter("out", [ROWS, DV], F32, isOutput=True)

    with tile.TileContext(nc) as tc:
        with (
            tc.tile_pool(name="dram", bufs=1, space="DRAM") as dram,
            tc.tile_pool(name="misc", bufs=1) as misc,
            tc.tile_pool(name="zt", bufs=1) as ztp,
            tc.tile_pool(name="expp", bufs=1) as expp,
            tc.tile_pool(name="wvp", bufs=1) as wvp,
            tc.tile_pool(name="qb", bufs=1) as qbp,
            tc.tile_pool(name="outp", bufs=4) as outp,
        ):
            # constants: full-width ones for the PE rowsum, exp bias; touch
            # Exp once so the ACT table set loads before the S phase
            ones128 = misc.tile([128, 128], BF16)
            nc.vector.memset(ones128[:], 1.0)
            bias_sb = misc.tile([128, 1], F32)
            nc.vector.memset(bias_sb[:], -SHIFT)
            warm_sb = misc.tile([128, 1], F32)
            nc.scalar.activation(warm_sb[:], bias_sb[:], Exp,
                                 bias=bias_sb[:], scale=1.0)

            # ---- resident loads ------------------------------------------
            # sync ring, strict need order: (wq_t, zT jchunk0_t) pairs so the
            # t-outer q projection starts after one pair; then WkT for B^T;
            # then the rest of zT for the S phase.
            ztv = d_zT.rearrange("(t p) n -> p t n", p=128)
            wqv = d_wq.rearrange("(t p) m -> p t m", p=128)
            wktv = d_wkt.rearrange("(t p) m -> p t m", p=128)
            wvv = d_wv.rearrange("(t p) m -> p t m", p=128)

            zt_sb = ztp.tile([128, DT, SEQ], F16)
            # Wv rides the gpsimd ring (needed only at the out phase); the
            # zn stream issues behind it there
            wv_sb = wvp.tile([128, DT, DV], F16)
            for t in range(DT):
                nc.gpsimd.dma_start(wv_sb[:, t, :], wvv[:, t, :])

            B_sb = qbp.tile([128, MT, ROWS], F16)
            expS = expp.tile([128, JT, ROWS], BF16)

            with (
                tc.tile_pool(name="wqk", bufs=1) as wqk,
                tc.tile_pool(name="ps_proj", bufs=8, space="PSUM") as psp,
            ):
                # the q pass needs ~290GB/s of (wq_t, zT_t) pairs — more
                # than one ring sustains — so wq rides the scalar ring in
                # parallel with zT on the sync ring
                wq_sb, wkt_sb = [], []
                for t in range(DT):
                    w = wqk.tile([128, DK], F16, name=f"wq{t}")
                    if t == 0:
                        # split so the very first matmul's operands (wq0
                        # m-slice 0 + own zT rows) land a few us earlier
                        nc.scalar.dma_start(w[:, 0:128], wqv[:, 0, 0:128])
                        nc.sync.dma_start(zt_sb[:, 0, 0:512], ztv[:, 0, 0:512])
                        nc.scalar.dma_start(w[:, 128:DK], wqv[:, 0, 128:DK])
                        nc.sync.dma_start(zt_sb[:, 0, 512:1024],
                                          ztv[:, 0, 512:1024])
                    else:
                        nc.scalar.dma_start(w[:], wqv[:, t, :])
                        nc.sync.dma_start(zt_sb[:, t, 0:1024],
                                          ztv[:, t, 0:1024])
                    wq_sb.append(w)
                for t in range(DT):
                    w = wqk.tile([128, D], F16, name=f"wkt{t}")
                    nc.sync.dma_start(w[:], wktv[:, t, :])
                    wkt_sb.append(w)
                for t in range(DT):
                    nc.sync.dma_start(zt_sb[:, t, 1024:SEQ],
                                      ztv[:, t, 1024:SEQ])

                # q^T[dk, r] = sum_t Wq[t-rows, dk-slice]^T zT[t-rows, own r]
                q_sb = wqk.tile([128, MT, ROWS], F16, name="qsb")
                psq = [psp.tile([128, ROWS], F32, tag="psp", name=f"psq{m}")
                       for m in range(MT)]
                for t in range(DT):
                    for m in range(MT):
                        nc.tensor.matmul(
                            psq[m][:], wq_sb[t][:, m * 128:(m + 1) * 128],
                            zt_sb[:, t, 0:ROWS],
                            start=(t == 0), stop=(t == DT - 1))
                # copies split across vector/scalar so the B phase's bank
                # reuse isn't gated on one engine draining 8 copies
                for m in range(MT):
                    eng = nc.vector.tensor_copy if m % 2 == 0 else nc.scalar.copy
                    eng(q_sb[:, m, :], psq[m][:])

                # B^T[d, r] = sum_dk Wk[d-slice, dk]^T q^T -> lhsT = WkT tiles
                # m-outer: all operands are resident by now, and each bank's
                # copy then hides under the next m group instead of stalling
                # the S phase on a burst of 8 copies
                psb = [psp.tile([128, ROWS], F32, tag="psp", name=f"psb{m}")
                       for m in range(MT)]
                for m in range(DT):
                    for t in range(MT):
                        nc.tensor.matmul(
                            psb[m][:], wkt_sb[t][:, m * 128:(m + 1) * 128],
                            q_sb[:, t, :],
                            start=(t == 0), stop=(t == MT - 1))
                    eng = nc.vector.tensor_copy if m % 2 == 0 else nc.scalar.copy
                    eng(B_sb[:, m, :], psb[m][:])

            # ---------------- S phase -------------------------------------
            # S^T[j, r] = sum_t zT[t, j-slice]^T B^T[t, r]; exp on ACT with
            # the -SHIFT bias; rowsum via ones-matmul accumulated across all
            # j into one persistent PSUM bank.
            # zn tiles for the C phase stream on the gpsimd ring; issue all
            # 32 up front — the 12-buf pool throttles the ring, which carries
            # nothing else.
            znv = d_zn.rearrange("(j p) m -> p j m", p=128)
            with (
                tc.tile_pool(name="znp", bufs=12) as znp,
                tc.tile_pool(name="csp", bufs=1) as csp,
            ):
                zn_sb = []
                for j in range(JT):
                    zn_t = znp.tile([128, D], F16, tag="zn", name=f"zn{j}")
                    nc.gpsimd.dma_start(zn_t[:], znv[:, j, :])
                    zn_sb.append(zn_t)

                mult_sb = misc.tile([128, ST], F32)
                with (
                    tc.tile_pool(name="ps_s", bufs=2, space="PSUM") as ps_s,
                    tc.tile_pool(name="ps_rs", bufs=1, space="PSUM") as ps_rs,
                ):
                    rs_ps = ps_rs.tile([128, ROWS], F32)
                    # rowsum for j is issued AFTER S j+1's matmuls: it waits
                    # on ACT's exp(j), which then overlaps S j+1 on the PE
                    def rowsum(j):
                        nc.tensor.matmul(rs_ps[:], ones128[:], expS[:, j, :],
                                         start=(j == 0), stop=(j == JT - 1))

                    for j in range(JT):
                        ps_S = ps_s.tile([128, ROWS], F32, tag="pss")
                        for t in range(DT):
                            nc.tensor.matmul(
                                ps_S[:], zt_sb[:, t, j * 128:(j + 1) * 128],
                                B_sb[:, t, :],
                                start=(t == 0), stop=(t == DT - 1))
                        nc.scalar.activation(expS[:, j, :], ps_S[:], Exp,
                                             bias=bias_sb[:], scale=1.0)
                        if j > 0:
                            rowsum(j - 1)
                    rowsum(JT - 1)

                    # row-sum -> per-row reciprocal multipliers [128, ST]
                    rs_sb = misc.tile([1, ROWS], F32)
                    nc.vector.tensor_copy(rs_sb[:], rs_ps[0:1, :])
                    rs_dram = dram.tile([1, ROWS], F32)
                    nc.scalar.dma_start(rs_dram[:], rs_sb[:])
                    rs128 = misc.tile([128, ST], F32)
                    nc.scalar.dma_start(
                        rs128[:], rs_dram[0, :].rearrange("(r p) -> p r",
                                                          p=128))
                    nc.vector.reciprocal(mult_sb[:], rs128[:])
                    nc.vector.tensor_scalar_mul(mult_sb[:], mult_sb[:], SCALE)

                # ---------------- C phase ---------------------------------
                # C^T[d, r] = sum_j zn[j, d-slice]^T P^T[j, r]
                C_sb = csp.tile([128, MT, ROWS], BF16)
                with tc.tile_pool(name="ps_c", bufs=8, space="PSUM") as ps_c:
                    psc = [ps_c.tile([128, ROWS], F32, tag="psc",
                                     name=f"psc{m}") for m in range(MT)]
                    for j in range(JT):
                        for m in range(MT):
                            nc.tensor.matmul(
                                psc[m][:], zn_sb[j][:, m * 128:(m + 1) * 128],
                                expS[:, j, :],
                                start=(j == 0), stop=(j == JT - 1))
                    for m in range(MT):
                        eng = (nc.vector.tensor_copy if m % 2 == 0
                               else nc.scalar.copy)
                        eng(C_sb[:, m, :], psc[m][:])

            # ---------------- out phase -----------------------------------
            # out[r-slice, e] = sum_m C^T[m, r-slice]^T Wv[m, e]
            with tc.tile_pool(name="ps_o", bufs=8, space="PSUM") as ps_o:
                for r in range(ST):
                    for h in range(2):
                        po = ps_o.tile([128, 512], F32, tag="po",
                                       name=f"po{r}{h}")
                        for m in range(MT):
                            nc.tensor.matmul(
                                po[:], C_sb[:, m, r * 128:(r + 1) * 128],
                                wv_sb[:, m, h * 512:(h + 1) * 512],
                                start=(m == 0), stop=(m == MT - 1))
                        o_sb = outp.tile([128, 512], F32, tag="osb")
                        nc.vector.tensor_scalar_mul(o_sb[:], po[:],
                                                    mult_sb[:, r:r + 1])
                        # alternate rings so the final writes drain in
                        # parallel instead of serializing the tail
                        deng = nc.sync if (r * 2 + h) % 2 == 0 else nc.scalar
                        deng.dma_start(
                            d_out[r * 128:(r + 1) * 128,
                                  h * 512:(h + 1) * 512],
                            o_sb[:])
    nc.compile()
    return nc


_BUILT = None


def make_in_maps(z, Wq, Wk, Wv):
    zT = np.ascontiguousarray(z.T).astype(np.float16)
    zn = z.astype(np.float16)
    wq16 = Wq.astype(np.float16)
    wkt16 = np.ascontiguousarray(Wk.T).astype(np.float16)
    wv16 = Wv.astype(np.float16)
    in_maps = []
    for c in range(NCORES):
        in_maps.append({
            "zT": np.ascontiguousarray(np.roll(zT, -c * ROWS, axis=1)),
            "zn": np.ascontiguousarray(np.roll(zn, -c * ROWS, axis=0)),
            "Wq": wq16,
            "WkT": wkt16,
            "Wv": wv16,
        })
    return in_maps


def kernel(z, Wq, Wk, Wv):
    global _BUILT
    from concourse.bass_utils import run_bass_kernel_spmd

    if _BUILT is None:
        _BUILT = _build()
    nc = _BUILT

    in_maps = make_in_maps(z, Wq, Wk, Wv)
    res = run_bass_kernel_spmd(nc, in_maps, list(range(NCORES)))
    out = np.concatenate([res.results[c]["out"] for c in range(NCORES)], axis=0)
    return out.astype(np.float32)


if __name__ == "__main__":
    rng = np.random.default_rng(0)
    z = rng.standard_normal((SEQ, D)).astype(np.float32)
    Wq = (0.02 * rng.standard_normal((D, DK))).astype(np.float32)
    Wk = (0.02 * rng.standard_normal((D, DK))).astype(np.float32)
    Wv = (0.02 * rng.standard_normal((D, DV))).astype(np.float32)
    out = kernel(z=z, Wq=Wq, Wk=Wk, Wv=Wv)
    print(out.shape, out.dtype)
